# revision 7
# baseline (speedup 1.0000x reference)
"""Trainium2 Bass kernel for a 2-layer LSTM text classifier.

Model: embedding lookup -> 2-layer BasicLSTM (H=100, T=25) -> dense(128)
-> dense(2). Batch 512 is data-parallel across 8 NeuronCores (64
rows/core); parameters replicated. The embedding gather happens
host-side (pure indexing); the two dense layers are linear so they fold
into a single [100, 2] head matrix host-side.

Device kernel: split-cell lagged wavefronts. Wavefront w runs layer-1
step w on the critical loop and layer-2 step w-2 entirely in its shadow:

  critical loop (layer 1):
    close z1[w] with W1h.h1[w-1] (4 MMs; W1x.x[w]+b1 pre-accumulated)
    tg1 = tanh(0.5 z1)        ACT   [100,256] from PSUM
    ph1 = (ti+1)*tj           DVE
    qh1 = (tf+1)*C1           DVE   (C = 2c doubled cell state)
    C1' = qh1*0.5 + ph1       DVE
    tc1 = tanh(0.5 C1')       ACT
    h1  = (to+1)*tc1          DVE   (= 2h; consumers fold the 0.5)
  shadow (layer 2, lag 2, z2 closed the wavefront before):
    tg2 (ACT), qh2/C2' (DVE), ph2/h2 (GpSimd tensor_tensor pairs),
    tc2 (ACT), then W2h.h2 closes next wavefront's z2 bank.

All gates go through tanh with a uniform ACT scale=0.5:
sigmoid(x) = (1+tanh(x/2))/2 for i/f/o; the j-gate weights carry a 2x
so tanh(0.5*2a) = tanh(a). Biases ride a ones-row at partition 100 of
the rhs tiles (xt, h tiles) paired with a bias row in the stationary
weights. Matmuls are bf16, 128-col stationaries (FWL), fp32 PSUM.

Inputs are two row-contiguous DMAs (xt, wpack) -- no small-element
descriptors (a [128,2] fp32 tensor costs ~30us of DMA-queue time).
"""

import functools
import os
import sys

import numpy as np

for _p in ("/opt/trn_rl_repo", "/root/.axon_site/_ro/trn_rl_repo"):
    if os.path.isdir(_p) and _p not in sys.path:
        sys.path.insert(0, _p)
        break

import ml_dtypes

from concourse import bass, bass2jax, mybir
from concourse.bass_utils import run_bass_kernel_spmd
from concourse.tile import TileContext

# --- BIR sync-wait rebalancer -------------------------------------------
# The walrus build in this image enforces ONE sync-wait command per ISA
# instruction struct, but Tile's semaphore assignment happily emits 2-4
# waits on matmuls/DVE ops at psum-recycle points. Rewrite the BIR before
# walrus: park one matmul wait on the adjacent Ldweights (same in-order
# queue, executes strictly before the matmul) and split any remaining
# excess onto pure-wait EventSemaphore carriers inserted directly before
# the offending instruction on its own queue. Semantics are unchanged --
# every wait still completes before the instruction it guarded.

_WAIT_PASSTHROUGH = {"EventSemaphore", "UnconditionalBranch", "Call",
                     "RegisterMove", "ISA"}


def _rebalance_bir_waits(bir_bytes):
    import orjson
    bir = orjson.loads(bir_bytes)
    n = 0
    for fn in bir["functions"]:
        for blk in fn["blocks"]:
            out = []
            prev = None
            for inst in blk["instructions"]:
                op = inst.get("opcode")
                si = inst.get("sync_info") or {}
                waits = si.get("on_wait") or []
                if op not in _WAIT_PASSTHROUGH and len(waits) > 1:
                    if (op == "Matmult" and prev is not None
                            and prev.get("opcode") == "Ldweights"
                            and not (prev.get("sync_info") or {}).get("on_wait")):
                        tsi = prev.setdefault("sync_info", {})
                        tsi.setdefault("on_wait", []).append(waits.pop(0))
                    while len(waits) > 1:
                        n += 1
                        out.append({
                            "debug": inst.get("debug", 0),
                            "engine": inst["engine"],
                            "ins": [], "outs": [],
                            "name": f"antwait_{n}",
                            "opcode": "EventSemaphore",
                            "sync_info": {"on_update": [],
                                          "on_wait": [waits.pop(0)]},
                        })
                    si["on_wait"] = waits
                out.append(inst)
                prev = inst
            blk["instructions"] = out
    return orjson.dumps(bir)


_orig_compile_bir_kernel = bass2jax.compile_bir_kernel


def _compile_bir_kernel_rebalanced(bir_json, tmpdir, neff_name="file.neff"):
    return _orig_compile_bir_kernel(_rebalance_bir_waits(bir_json), tmpdir,
                                    neff_name=neff_name)


if bass2jax.compile_bir_kernel is not _compile_bir_kernel_rebalanced:
    bass2jax.compile_bir_kernel = _compile_bir_kernel_rebalanced

H = 100          # hidden size
T = 25           # sequence length
B = 512          # total batch
N_CORES = 8
BC = B // N_CORES  # 64 per-core batch
NCLS = 2         # logits
FORGET_BIAS = 1.0
LAG = 2          # layer-2 wavefront lag
W_TOT = T + LAG  # 27 wavefronts
TX = W_TOT       # xt columns (t=25,26 zero-padded)

BF16 = ml_dtypes.bfloat16
_DT = mybir.dt
TANH = mybir.ActivationFunctionType.Tanh
ADD = mybir.AluOpType.add
MULT = mybir.AluOpType.mult

# per-layer z-bank / tg slot layout: [i f j o] x 64 cols
_GATES = ("i", "f", "j", "o")
_SLOT = {"i": 0, "f": 64, "j": 128, "o": 192}

# wpack column layout (bf16, [101, 2560]):
_W1X, _W1H, _W2X, _W2H = 0, 512, 1024, 1536
_WEFF = 2048            # [101, 2] fused fc1@fc2 head (+bias row)
_HINIT = 2176           # [101, 384] zeros + ones row -> h tile init


def _build_nc():
    nc = bass.Bass()
    xt_d = nc.dram_tensor("xt", [H + 1, TX * BC], _DT.bfloat16,
                          kind="ExternalInput")
    wp_d = nc.dram_tensor("wpack", [H + 1, 2560], _DT.bfloat16,
                          kind="ExternalInput")
    out_d = nc.dram_tensor("out", [NCLS, BC], _DT.float32, kind="ExternalOutput")

    with TileContext(nc) as tc:
        with tc.tile_pool(name="const", bufs=1) as cpool, \
             tc.tile_pool(name="work", bufs=2) as wpool, \
             tc.tile_pool(name="ps1", bufs=3, space="PSUM") as z1pool, \
             tc.tile_pool(name="ps2", bufs=3, space="PSUM") as z2pool, \
             tc.tile_pool(name="psh", bufs=1, space="PSUM") as hpool:

            # warm the tanh table on ACT while DMAs run
            scratch = cpool.tile([1, 1], _DT.float32, tag="scratch")
            nc.vector.memset(scratch[:, :], 0.0)
            nc.scalar.activation(scratch[:, :], scratch[:, :], TANH)

            xt = cpool.tile([H + 1, TX * BC], _DT.bfloat16, tag="xt")
            wp = cpool.tile([H + 1, 2560], _DT.bfloat16, tag="wp")
            nc.sync.dma_start(out=wp[:, :], in_=wp_d[:, :])
            nc.scalar.dma_start(out=xt[:, :], in_=xt_d[:, :])

            w1x = wp[:, _W1X:_W1X + 512]
            w1h = wp[0:H, _W1H:_W1H + 512]
            w2x = wp[:, _W2X:_W2X + 512]
            w2h = wp[0:H, _W2H:_W2H + 512]
            weff = wp[:, _WEFF:_WEFF + NCLS]

            # h tiles: slot k%3 holds the h written at wavefront k
            # (h1[k] / h2[k-2]), [101, 64] each with a ones row at
            # partition 100 (from hinit, never rewritten).
            h1a = cpool.tile([H + 1, 192], _DT.bfloat16, tag="h1a")
            h2a = cpool.tile([H + 1, 192], _DT.bfloat16, tag="h2a")
            nc.vector.tensor_copy(h1a[:, :], wp[:, _HINIT:_HINIT + 192])
            nc.vector.tensor_copy(h2a[:, :], wp[:, _HINIT + 192:_HINIT + 384])
            c1a = cpool.tile([H, 128], _DT.float32, tag="c1a")
            c2a = cpool.tile([H, 128], _DT.float32, tag="c2a")
            nc.vector.memset(c1a[:, :], 0.0)
            nc.gpsimd.memset(c2a[:, :], 0.0)

            def h1s(k):
                return h1a[:, (k % 3) * 64:(k % 3) * 64 + 64]

            def h2s(k):  # h2[k-2], written at wavefront k
                return h2a[:, (k % 3) * 64:(k % 3) * 64 + 64]

            def mm4(z, wmat, rhs, start, stop):
                for n, g in enumerate(_GATES):
                    nc.tensor.matmul(
                        z[0:128, _SLOT[g]:_SLOT[g] + 64],
                        lhsT=wmat[:, n * 128:n * 128 + 128],
                        rhs=rhs,
                        start=(start and n == 0), stop=(stop and n == 3))

            # prologue: open+close z1[0] opener half and z2 bank 0 fully
            z1 = z1pool.tile([128, 256], _DT.float32, tag="z1")
            mm4(z1, w1x, xt[0:H + 1, 0:BC], True, False)
            z2 = z2pool.tile([128, 256], _DT.float32, tag="z2")
            mm4(z2, w2x, h1s(-2)[0:H + 1, :], True, False)
            mm4(z2, w2h, h2s(-1)[0:H, :], False, True)

            for w in range(W_TOT):
                # ---- critical loop: close z1[w], cell 1 ----
                mm4(z1, w1h, h1s(w - 1)[0:H, :], False, True)

                z1n = z2n = None
                if w + 1 < W_TOT:
                    z1n = z1pool.tile([128, 256], _DT.float32, tag="z1")
                    mm4(z1n, w1x, xt[0:H + 1, (w + 1) * BC:(w + 2) * BC],
                        True, False)
                    z2n = z2pool.tile([128, 256], _DT.float32, tag="z2")
                    mm4(z2n, w2x, h1s(w - 1)[0:H + 1, :], True, False)

                tg1 = wpool.tile([128, 256], _DT.bfloat16, tag="tg1",
                                 name=f"tg1_{w % 2}")
                nc.scalar.activation(tg1[0:H, :], z1[0:H, :], TANH, scale=0.5)
                c1r = c1a[:, ((w + 1) % 2) * 64:((w + 1) % 2) * 64 + 64]
                c1w = c1a[:, (w % 2) * 64:(w % 2) * 64 + 64]
                ph1 = wpool.tile([H, 64], _DT.bfloat16, tag="ph1",
                                 name=f"ph1_{w % 2}")
                nc.vector.scalar_tensor_tensor(
                    ph1[:, :], tg1[0:H, 0:64], 1.0, tg1[0:H, 128:192],
                    op0=ADD, op1=MULT)
                qh1 = wpool.tile([H, 64], _DT.float32, tag="qh1",
                                 name=f"qh1_{w % 2}")
                nc.vector.scalar_tensor_tensor(
                    qh1[:, :], tg1[0:H, 64:128], 1.0, c1r, op0=ADD, op1=MULT)
                nc.vector.scalar_tensor_tensor(
                    c1w, qh1[:, :], 0.5, ph1[:, :], op0=MULT, op1=ADD)
                tc1 = wpool.tile([H, 64], _DT.bfloat16, tag="tc1",
                                 name=f"tc1_{w % 2}")
                nc.scalar.activation(tc1[:, :], c1w, TANH, scale=0.5)
                nc.vector.scalar_tensor_tensor(
                    h1s(w)[0:H, :], tg1[0:H, 192:256], 1.0, tc1[:, :],
                    op0=ADD, op1=MULT)

                # ---- shadow: cell 2 on z2 (= z2[w-2], closed last wf) ----
                tg2 = wpool.tile([128, 256], _DT.bfloat16, tag="tg2",
                                 name=f"tg2_{w % 2}")
                nc.scalar.activation(tg2[0:H, :], z2[0:H, :], TANH, scale=0.5)
                c2r = c2a[:, ((w + 1) % 2) * 64:((w + 1) % 2) * 64 + 64]
                c2w = c2a[:, (w % 2) * 64:(w % 2) * 64 + 64]
                # ph2 = (ti+1)*tj and h2 = (to+1)*tc2 run on GpSimd as
                # tensor_tensor pairs (stt unsupported there) to keep
                # the DVE queue clear for the critical loop.
                p2m = wpool.tile([H, 64], _DT.bfloat16, tag="p2m",
                                 name=f"p2m_{w % 2}")
                nc.gpsimd.tensor_tensor(p2m[:, :], tg2[0:H, 0:64],
                                        tg2[0:H, 128:192], op=MULT)
                ph2 = wpool.tile([H, 64], _DT.bfloat16, tag="ph2",
                                 name=f"ph2_{w % 2}")
                nc.gpsimd.tensor_tensor(ph2[:, :], p2m[:, :],
                                        tg2[0:H, 128:192], op=ADD)
                qh2 = wpool.tile([H, 64], _DT.float32, tag="qh2",
                                 name=f"qh2_{w % 2}")
                nc.vector.scalar_tensor_tensor(
                    qh2[:, :], tg2[0:H, 64:128], 1.0, c2r, op0=ADD, op1=MULT)
                nc.vector.scalar_tensor_tensor(
                    c2w, qh2[:, :], 0.5, ph2[:, :], op0=MULT, op1=ADD)
                tc2 = wpool.tile([H, 64], _DT.bfloat16, tag="tc2",
                                 name=f"tc2_{w % 2}")
                nc.scalar.activation(tc2[:, :], c2w, TANH, scale=0.5)
                h2m = wpool.tile([H, 64], _DT.bfloat16, tag="h2m",
                                 name=f"h2m_{w % 2}")
                nc.gpsimd.tensor_tensor(h2m[:, :], tg2[0:H, 192:256],
                                        tc2[:, :], op=MULT)
                nc.gpsimd.tensor_tensor(h2s(w)[0:H, :], h2m[:, :],
                                        tc2[:, :], op=ADD)
                if w < 2:
                    # true h2[w-2]/C2[w-2] are zero (pre-sequence state)
                    nc.gpsimd.memset(h2s(w)[0:H, :], 0.0)
                    if w == 1:
                        nc.gpsimd.memset(c2w, 0.0)

                # close next z2 bank with W2h.h2[w-2] (just produced)
                if z2n is not None:
                    mm4(z2n, w2h, h2s(w)[0:H, :], False, True)

                z1, z2 = z1n, z2n

            # classifier head: pred = h2[T-1] @ weff (+bias via ones row)
            predp = hpool.tile([128, BC], _DT.float32, tag="pred", name="predp")
            nc.tensor.matmul(predp[0:NCLS, :], lhsT=weff[0:H + 1, 0:NCLS],
                             rhs=h2a[0:H + 1, ((W_TOT - 1) % 3) * 64:
                                     ((W_TOT - 1) % 3) * 64 + 64],
                             start=True, stop=True)
            outs = wpool.tile([NCLS, BC], _DT.float32, tag="outs")
            nc.vector.tensor_copy(outs[:, :], predp[0:NCLS, :])
            nc.sync.dma_start(out=out_d[:, :], in_=outs[:, :])

    return nc


@functools.lru_cache(maxsize=1)
def _get_nc():
    return _build_nc()


def _gate_cols(kmat, rows, h_scale):
    """[rows, 512] fp32: 4 gate blocks of 128 cols (100 used) in i,f,j,o
    order from TF's (i,j,f,o), j doubled (tanh trick), scaled h_scale."""
    src = {"i": 0, "j": 1, "f": 2, "o": 3}
    out = np.zeros((rows.stop - rows.start, 512), np.float32)
    for n, g in enumerate(_GATES):
        s = h_scale * (2.0 if g == "j" else 1.0)
        out[:, n * 128:n * 128 + H] = kmat[rows, src[g] * H:(src[g] + 1) * H] * s
    return out


def _bias_row(bvec):
    src = {"i": 0, "j": 1, "f": 2, "o": 3}
    out = np.zeros(512, np.float32)
    for n, g in enumerate(_GATES):
        b = bvec[src[g] * H:(src[g] + 1) * H].copy()
        if g == "f":
            b += FORGET_BIAS
        if g == "j":
            b *= 2.0
        out[n * 128:n * 128 + H] = b
    return out


def _prep_weights(k1, b1, k2, b2, w_fc1, b_fc1, w_fc2, b_fc2):
    wpack = np.zeros((H + 1, 2560), np.float32)
    wpack[0:H, _W1X:_W1X + 512] = _gate_cols(k1, slice(0, H), 1.0)
    wpack[H, _W1X:_W1X + 512] = _bias_row(b1)
    # W1h, W2x, W2h consume doubled h -> 0.5
    wpack[0:H, _W1H:_W1H + 512] = _gate_cols(k1, slice(H, 2 * H), 0.5)
    wpack[0:H, _W2X:_W2X + 512] = _gate_cols(k2, slice(0, H), 0.5)
    wpack[H, _W2X:_W2X + 512] = _bias_row(b2)
    wpack[0:H, _W2H:_W2H + 512] = _gate_cols(k2, slice(H, 2 * H), 0.5)
    # fused linear head: pred = h2 @ (w_fc1 @ w_fc2) + (b_fc1 @ w_fc2 + b_fc2)
    weff = (w_fc1.astype(np.float64) @ w_fc2.astype(np.float64))
    beff = (b_fc1.astype(np.float64) @ w_fc2.astype(np.float64)
            + b_fc2.astype(np.float64))
    wpack[0:H, _WEFF:_WEFF + NCLS] = 0.5 * weff.astype(np.float32)
    wpack[H, _WEFF:_WEFF + NCLS] = beff.astype(np.float32)
    # hinit: zeros + ones row for the h1/h2 tile arrays
    wpack[H, _HINIT:_HINIT + 384] = 1.0
    return {"wpack": wpack.astype(BF16)}


def _run(inputs, trace=False):
    nc = _get_nc()
    feats = np.asarray(inputs["features"])
    x = np.asarray(inputs["embedding"])[feats]          # [B, T, H] host gather
    shared = _prep_weights(
        np.asarray(inputs["k1"]), np.asarray(inputs["b1"]),
        np.asarray(inputs["k2"]), np.asarray(inputs["b2"]),
        np.asarray(inputs["w_fc1"]), np.asarray(inputs["b_fc1"]),
        np.asarray(inputs["w_fc2"]), np.asarray(inputs["b_fc2"]))
    in_maps = []
    for c in range(N_CORES):
        xt = np.zeros((H + 1, TX * BC), np.float32)
        xt[0:H, 0:T * BC] = (
            x[c * BC:(c + 1) * BC].transpose(2, 1, 0).reshape(H, T * BC))
        xt[H, :] = 1.0
        in_maps.append({**shared, "xt": xt.astype(BF16)})
    res = run_bass_kernel_spmd(nc, in_maps, core_ids=list(range(N_CORES)),
                               trace=trace)
    out = np.empty((B, NCLS), np.float32)
    for c in range(N_CORES):
        out[c * BC:(c + 1) * BC] = res.results[c]["out"].T
    return out, res


def kernel(**inputs):
    out, _ = _run(inputs, trace=False)
    return out
